# revision 31
# baseline (speedup 1.0000x reference)
"""Multi-head causal attention (B=2, S=2048, D=1024, H=16, DK=64) on 8 trn2 cores.

Sharding: 2-way data parallel over batch x 4-way tensor parallel over heads.
Core c handles batch b = c // 4 and head group hg = c % 4 (4 heads = 256 dims).

Per-core kernel (all in transposed "feature on partitions" layouts, bf16
matmul operands, fp32 PSUM accumulation):
  QT[d2, n] = Wq^T X projection accumulated over D in PSUM; KT likewise; V in
  natural [n, d] layout augmented with a trailing ones column so the attnV
  matmul also produces the softmax denominator on PSUM row 64.
  Scores are computed directly transposed: S^T[k, q] = K_tile @ Q (contract
  over head dim), exp'd on ACT (no max subtraction: scores are O(5) so exp
  cannot overflow), causal-masked by zeroing the diagonal block's upper
  triangle with a gpsimd affine_select (Pool engine, off the DVE), then
  O^T[d, q] = sum_t Vaug_t^T @ exp(S^T_t) accumulated in PSUM.
  The denominator row is reciprocal'd (DVE, 1-partition), broadcast to 64
  partitions on gpsimd, and the normalization writes straight into the
  head's half of the paired O^T layout (the DVE output crossbar handles the
  64-partition shift for odd heads).  The output projection contracts over
  local head dims per 512-column PSUM block, staged through SBUF to DRAM.
  Host sums the 4 head-group partials per batch and adds the bias.

Schedule: a wavefront over 512-query superblocks.  Attention for superblock
r is the primary stream; interleaved aux work is balanced against each
round's ACT load: r0/r1 carry the next rounds' X projections (PE/DMA
heavy), r2 carries the last projections plus out-proj of r0, and the
ACT-heavy r3 carries out-proj of r1+r2 and fuses exp over pairs of
off-diagonal k-tiles (two adjacent PSUM banks per activation call) to cut
ACT call overhead where ACT is the binding engine.
"""

import numpy as np

B, S, D, DK = 2, 2048, 1024, 64
H = D // DK  # 16
NCORES = 8
BATCH_SHARDS = 2
HEAD_SHARDS = 4
HL = H // HEAD_SHARDS  # heads per core
DL = HL * DK  # local head dims per core

import os as _os

_DT = _os.environ.get("MHA_DT", "bf16")  # bf16 | f32r | f32 matmul operands
_R3_PAIR = _os.environ.get("MHA_R3_PAIR", "1") == "1"


def build_nc(s_core=S, d_model=D, hl=HL, dt=_DT):
    from contextlib import ExitStack

    import concourse.bacc as bacc
    import concourse.bass as bass
    import concourse.mybir as mybir
    import concourse.tile as tile

    f32 = mybir.dt.float32
    mdt = {
        "bf16": mybir.dt.bfloat16,
        "f32r": mybir.dt.float32r,
        "f32": f32,
    }[dt]
    Exp = mybir.ActivationFunctionType.Exp

    dl = hl * DK
    nhb = max(1, dl // 128)  # 128-wide blocks of local head dims
    KB = d_model // 128  # contraction tiles for projections
    NT = s_core // 128  # token tiles
    QSB = 512  # query superblock
    NQSB = s_core // QSB

    nc = bacc.Bacc("TRN2", target_bir_lowering=False, debug=False)
    xq = nc.declare_dram_parameter("xq", [d_model, s_core], mdt, isOutput=False)
    xk = nc.declare_dram_parameter("xk", [d_model, s_core], mdt, isOutput=False)
    xv = nc.declare_dram_parameter("xv", [d_model, s_core], mdt, isOutput=False)
    wq = nc.declare_dram_parameter("wq", [d_model, dl], mdt, isOutput=False)
    wk = nc.declare_dram_parameter("wk", [d_model, dl], mdt, isOutput=False)
    wv = nc.declare_dram_parameter("wv", [d_model, dl], mdt, isOutput=False)
    wp = nc.declare_dram_parameter("wp", [dl, d_model], mdt, isOutput=False)
    odt = mybir.dt.bfloat16 if dt == "bf16" else f32
    out = nc.declare_dram_parameter("out", [s_core, d_model], odt, isOutput=True)

    with ExitStack() as ctx:
        tc = ctx.enter_context(tile.TileContext(nc))
        sb = ctx.enter_context(tc.tile_pool(name="sb", bufs=1))
        stream = ctx.enter_context(tc.tile_pool(name="stream", bufs=2))
        work = ctx.enter_context(tc.tile_pool(name="work", bufs=3))
        psum = ctx.enter_context(tc.tile_pool(name="psum", bufs=8, space="PSUM"))

        # PSUM budget (8 banks): 3 double-bank score tiles keep the
        # score->exp->attnV pipeline 2 units deep; the remaining 2 banks
        # rotate between the AV accumulator (released promptly by the
        # epilogue's PSUM->SBUF copy) and whichever aux generator is active.
        def bank():
            return psum.tile([128, 512], f32, tag="bank", bufs=2, name="bank")

        def bank2():
            return psum.tile([128, 2, 512], f32, tag="bank2", bufs=3, name="bank2")

        # ---- persistent SBUF state ----
        wq_sb = sb.tile([128, KB, dl], mdt)
        wk_sb = sb.tile([128, KB, dl], mdt)
        wv_sb = sb.tile([128, KB, dl], mdt)

        def load_w(wsb, wdr, i):
            # one tensor's half of the contraction dim: 256KB per DMA keeps
            # the transfer (728ns) above the serial HWDGE issue cost (625ns)
            KQ = KB // 2
            nc.sync.dma_start(
                out=wsb[:, i * KQ : (i + 1) * KQ, :],
                in_=wdr[i * KQ * 128 : (i + 1) * KQ * 128, :].rearrange(
                    "(kb p) m -> p kb m", p=128
                ),
            )

        wp_sb = sb.tile([128, hl // 2, d_model], mdt)

        qt_sb = sb.tile([128, nhb, s_core], mdt)  # [d2, hb, n]
        kt_sb = sb.tile([128, nhb, s_core], mdt)
        vaug = sb.tile([128, hl, NT, DK + 1], mdt)  # [k, h, ktile, [d | 1]]
        ones_col = vaug[:, :, :, DK : DK + 1]
        if mdt == mybir.dt.float32r:
            ones_col = ones_col.bitcast(f32)
        nc.vector.memset(ones_col, 1.0)
        ot_sb = sb.tile([128, hl // 2, s_core], mdt)  # [(h%2)*64+d, hp, n]

        KH = KB // 2  # stream X in two half-contraction tiles per chunk

        def x_dma(nb, xi, kh):
            src = (xq, xk, xv)[xi]
            t = stream.tile([128, KH, 256], mdt, tag=f"x{xi}{kh}", name=f"x{xi}{kh}")
            r0 = kh * KH * 128
            nc.sync.dma_start(
                out=t,
                in_=src[r0 : r0 + KH * 128, nb * 256 : nb * 256 + 256].rearrange(
                    "(kb p) n -> p kb n", p=128
                ),
            )
            return t

        def proj_qk_units(nb, fine=False):
            """Generator: Q and K projections for one 256-token chunk of X.
            Yields between small instruction groups so the driver can
            interleave this PE/DMA-heavy work into the attention stream.
            fine=True yields after every X DMA so the startup can sequence
            loads in critical-path order."""
            n0 = nb * 256
            xts = {}
            for xi in range(2):
                for kh in range(2):
                    xts[(xi, kh)] = x_dma(nb, xi, kh)
                    if fine:
                        yield
            if not fine:
                yield
            # Q then K, one head-block (= one PSUM bank) at a time
            for dst, xi in ((qt_sb, 0), (kt_sb, 1)):
                wsb = (wq_sb, wk_sb)[xi]
                for hb in range(nhb):
                    ps = bank()
                    for kb in range(KB):
                        kh, kbl = divmod(kb, KH)
                        nc.tensor.matmul(
                            ps[:, :256],
                            wsb[:, kb, hb * 128 : hb * 128 + 128],
                            xts[(xi, kh)][:, kbl, :],
                            start=kb == 0,
                            stop=kb == KB - 1,
                        )
                        if kb % 2 == 1:
                            yield
                    nc.vector.tensor_copy(
                        out=dst[:, hb, n0 : n0 + 256], in_=ps[:, :256]
                    )
                    yield

        def proj_v_units(nb, fine=False):
            """Generator: V projection (vaug) for one 256-token chunk."""
            xvts = {}
            for kh in range(2):
                xvts[kh] = x_dma(nb, 2, kh)
                if fine:
                    yield
            if not fine:
                yield
            for j in range(2):  # two 128-token tiles per chunk
                nt = nb * 2 + j
                ps_v = bank()
                for kb in range(KB):
                    kh, kbl = divmod(kb, KH)
                    nc.tensor.matmul(
                        ps_v[:, :dl],
                        xvts[kh][:, kbl, j * 128 : j * 128 + 128],
                        wv_sb[:, kb, :],
                        start=kb == 0,
                        stop=kb == KB - 1,
                    )
                    if kb % 2 == 1:
                        yield
                nc.vector.tensor_copy(
                    out=vaug[:, :, nt, 0:DK],
                    in_=ps_v[:, :dl].rearrange("p (h d) -> p h d", d=DK),
                )
                yield

        N_QK_UNITS = 1 + nhb * 2 * 5  # non-fine yield count
        N_V_UNITS = 1 + 2 * 5

        def att_units(qsb, pair=False, tail=False):
            """Generator: causal attention for all heads of one query
            superblock.  Units are (head, k-tile) — or, with pair=True,
            (head, two off-diagonal k-tiles sharing one double-bank PSUM
            tile and one exp call).  The unit two steps ahead is always in
            flight so the PE never drains waiting on exp."""
            q0 = qsb * QSB
            nkt = (qsb + 1) * (QSB // 128)
            ndiag = QSB // 128
            pos = {}
            ets = {}

            units = []
            for h in range(hl):
                t = 0
                while t < nkt:
                    if pair and t + 1 < nkt - ndiag:
                        units.append(("p", h, t))
                        t += 2
                    else:
                        units.append(("s", h, t))
                        t += 1

            def epilogue(h, po, last=False):
                # denominator -> reciprocal (1-partition DVE op) and an
                # eager PSUM->SBUF copy (releases the accumulator bank fast,
                # keeping the 2-slot "bank" ring from stalling the next
                # head), then gpsimd partition broadcast -> normalize.  The
                # DVE output crossbar shifts odd heads' 64 partitions up.
                # The kernel's very last epilogue skips the eager copy and
                # runs in two half-column chunks instead, so the output
                # projection of the first token tiles starts ~1.5us sooner.
                hp, ho = h // 2, (h % 2) * 64
                for cl, cr in ((0, QSB // 2), (QSB // 2, QSB)) if last else ((0, QSB),):
                    recip_q = work.tile(
                        [1, QSB], f32, tag="recip", bufs=3, name="recip_q"
                    )
                    nc.vector.reciprocal(out=recip_q[:, cl:cr], in_=po[64:65, cl:cr])
                    if last:
                        pox = po[0:64, :]
                    else:
                        pox = work.tile([64, QSB], f32, tag="pox", bufs=2, name="pox")
                        nc.vector.tensor_copy(out=pox, in_=po[0:64, :])
                    rb = work.tile([64, QSB], f32, tag="rb", bufs=3, name="rb")
                    nc.gpsimd.partition_broadcast(
                        out_ap=rb[:, cl:cr], in_ap=recip_q[:, cl:cr]
                    )
                    nc.vector.tensor_mul(
                        out=ot_sb[ho : ho + 64, hp, q0 + cl : q0 + cr],
                        in0=pox[:, cl:cr],
                        in1=rb[:, cl:cr],
                    )

            def mask_diag(et, c0):
                # zero the upper (k > q) triangle of the diagonal 128-block
                nc.gpsimd.affine_select(
                    out=et[:, c0 : c0 + 128],
                    in_=et[:, c0 : c0 + 128],
                    compare_op=mybir.AluOpType.is_ge,
                    fill=0.0,
                    base=0,
                    pattern=[[1, 128]],
                    channel_multiplier=-1,
                )

            def kq_mm(dst, h, t, c0):
                hb, ho = h // 2, (h % 2) * 64
                nc.tensor.matmul(
                    dst,
                    kt_sb[ho : ho + 64, hb, t * 128 : t * 128 + 128],
                    qt_sb[ho : ho + 64, hb, q0 + c0 : q0 + QSB],
                    start=True,
                    stop=True,
                )

            def score(u):
                kind, h, t = u
                if kind == "p":
                    ps = bank2()
                    kq_mm(ps[:, 0, :], h, t, 0)
                    kq_mm(ps[:, 1, :], h, t + 1, 0)
                    et = work.tile(
                        [128, 2, QSB], mdt, tag="et2", bufs=4, name="et2"
                    )
                    nc.scalar.activation(out=et, in_=ps, func=Exp)
                    ets[u] = et
                else:
                    r = t - qsb * (QSB // 128)
                    c0 = r * 128 if r > 0 else 0
                    ps = bank2()[:, 0, :]
                    kq_mm(ps[:, c0:QSB], h, t, c0)
                    et = work.tile([128, QSB], mdt, tag="et", bufs=6, name="et")
                    nc.scalar.activation(out=et[:, c0:QSB], in_=ps[:, c0:QSB], func=Exp)
                    if r >= 0:
                        mask_diag(et, c0)
                    ets[u] = et

            def av_one(h, t, rhs, c0):
                if t == 0:
                    pos[h] = bank()
                nc.tensor.matmul(
                    pos[h][0:65, c0:QSB],
                    vaug[:, h, t, :],
                    rhs,
                    start=t == 0,
                    stop=t == nkt - 1,
                )
                if t == nkt - 1:
                    epilogue(h, pos.pop(h), last=tail and h == hl - 1)

            def av(u):
                kind, h, t = u
                et = ets.pop(u)
                if kind == "p":
                    av_one(h, t, et[:, 0, :], 0)
                    av_one(h, t + 1, et[:, 1, :], 0)
                else:
                    r = t - qsb * (QSB // 128)
                    c0 = r * 128 if r > 0 else 0
                    av_one(h, t, et[:, c0:QSB], c0)

            for u_i, u in enumerate(units):
                score(u)
                if u_i >= 2:
                    av(units[u_i - 2])
                yield
            for u in units[-2:]:
                av(u)

        def n_att_units(qsb, pair=False):
            nkt = (qsb + 1) * (QSB // 128)
            noff = nkt - QSB // 128
            if pair:
                return hl * (noff // 2 + QSB // 128)
            return hl * nkt

        def out_units(qsb):
            """Generator: output projection for one query superblock."""
            for nt in range(qsb * 4, qsb * 4 + 4):
                for cb in range(d_model // 512):
                    p3 = bank()
                    for hp in range(hl // 2):
                        nc.tensor.matmul(
                            p3,
                            ot_sb[:, hp, nt * 128 : nt * 128 + 128],
                            wp_sb[:, hp, cb * 512 : cb * 512 + 512],
                            start=hp == 0,
                            stop=hp == hl // 2 - 1,
                        )
                    os_t = work.tile([128, 512], odt, tag="osb", bufs=4, name="os_t")
                    nc.vector.tensor_copy(out=os_t, in_=p3)
                    nc.sync.dma_start(
                        out=out[nt * 128 : nt * 128 + 128, cb * 512 : cb * 512 + 512],
                        in_=os_t,
                    )
                    yield

        def out_tail(qsb):
            """Output projection for the final superblock.  The first head
            pair's partial products run during the last head's epilogue
            (their inputs were ready mid-round); only the second pair's
            matmuls, the split DVE/ACT staging copies, and one DMA per token
            tile sit on the critical tail.  Three double-bank tiles cover
            the first three token tiles; the fourth waits for the first
            tile's copies so the emission order stays deadlock-free."""

            def hp_mms(p3, nt, hp):
                for cb in range(d_model // 512):
                    nc.tensor.matmul(
                        p3[:, cb, :],
                        ot_sb[:, hp, nt * 128 : nt * 128 + 128],
                        wp_sb[:, hp, cb * 512 : cb * 512 + 512],
                        start=hp == 0,
                        stop=hp == hl // 2 - 1,
                    )

            def finish(p3, nt):
                os_row = work.tile(
                    [128, d_model], odt, tag="osb2", bufs=4, name="os_row"
                )
                nc.vector.tensor_copy(out=os_row[:, 0:512], in_=p3[:, 0, :])
                nc.scalar.copy(out=os_row[:, 512:1024], in_=p3[:, 1, :])
                nc.sync.dma_start(out=out[nt * 128 : nt * 128 + 128, :], in_=os_row)

            class BankPair:
                # two single banks presented with the double-bank interface,
                # so the 4th token tile's partials can start on the "bank"
                # ring (free once the last AV accumulator is copied out)
                # without waiting for a bank2 slot.
                def __init__(self):
                    self.b = [bank(), bank()]

                def __getitem__(self, idx):
                    return self.b[idx[1]][:, idx[2]]

            nts = list(range(qsb * 4, qsb * 4 + 4))
            p3s = {}
            for nt in nts[:3]:
                p3s[nt] = bank2()
                hp_mms(p3s[nt], nt, 0)
            p3s[nts[3]] = BankPair()
            hp_mms(p3s[nts[3]], nts[3], 0)
            for nt in nts:
                hp_mms(p3s[nt], nt, 1)
                finish(p3s.pop(nt), nt)

        def drain(gen):
            for _ in gen:
                pass

        def chain(*gens):
            for g in gens:
                yield from g

        # ---- PE warm-up: the tensor engine needs ~3us of continuous work
        # to reach its top p-state; the first real matmul can't start until
        # weights+activations land (~4us of DMA).  A train of dependency-free
        # dummy matmuls covers the ramp so real work runs at full rate.
        warm = sb.tile([1, 512], mdt, name="warm")
        nc.vector.memset(warm.bitcast(f32) if mdt == mybir.dt.float32r else warm, 1.0)
        wps = bank2()
        for _ in range(7):
            nc.tensor.matmul(wps[0:1, 0, :], warm[:, 0:1], warm, start=True, stop=True)

        # ---- startup: Q/K projections of chunks 0,1 with strict DMA
        # priority on their critical path; V loads queue behind them and the
        # V projections fold into round 0's aux stream (the whole
        # startup+round-0 phase is DMA-bandwidth-bound).
        gq0, gq1 = proj_qk_units(0, fine=True), proj_qk_units(1)
        gv0, gv1 = proj_v_units(0, fine=True), proj_v_units(1, fine=True)
        next(gq0)  # xq chunk0 kh0
        load_w(wq_sb, wq, 0)
        next(gq0)  # xq chunk0 kh1
        load_w(wq_sb, wq, 1)
        next(gq0)  # xk chunk0 kh0
        load_w(wk_sb, wk, 0)
        next(gq0)  # xk chunk0 kh1
        load_w(wk_sb, wk, 1)
        next(gq1)  # chunk 1 xq+xk loads
        next(gv0)  # xv chunk0 kh0
        load_w(wv_sb, wv, 0)
        next(gv0)  # xv chunk0 kh1
        load_w(wv_sb, wv, 1)
        next(gv1)  # xv chunk1 kh0
        next(gv1)  # xv chunk1 kh1
        drain(gq0)
        drain(gq1)
        nc.sync.dma_start(out=wp_sb, in_=wp[:, :].rearrange("(hp x) c -> x hp c", x=128))

        # ---- wavefront rounds: aux placement balances PE against each
        # round's ACT load (out-proj deferred to the ACT-heavy late rounds;
        # chunk-0/1 V projections lead round 0 so the first AVs are fed)
        out_round = {2: [0], 3: [1, 2]}
        for qsb in range(NQSB):
            aux_gens = []
            n_aux = 0
            if qsb == 0:
                aux_gens += [gv0, gv1]
                n_aux += 2 * 10
            if qsb + 1 < NQSB:
                for c in (2 * qsb + 2, 2 * qsb + 3):
                    aux_gens += [proj_qk_units(c), proj_v_units(c)]
                n_aux += 2 * (N_QK_UNITS + N_V_UNITS)
            for oq in out_round.get(qsb, []):
                aux_gens.append(out_units(oq))
                n_aux += 8
            aux = chain(*aux_gens)
            pair = _R3_PAIR
            n_att = n_att_units(qsb, pair)
            rate = _os.environ.get("MHA_AUX_SHAPE", "even")
            acc = 0.0
            for i, _ in enumerate(att_units(qsb, pair, tail=qsb == NQSB - 1)):
                if rate == "front":
                    acc += (n_aux / n_att) * (1.5 if i < n_att // 2 else 0.5)
                else:
                    acc += n_aux / n_att
                while acc >= 1.0:
                    acc -= 1.0
                    next(aux, None)
            drain(aux)
        out_tail(NQSB - 1)

    nc.compile()
    return nc


_NC_CACHE = {}


def _get_nc():
    key = (S, D, HL, _DT, _R3_PAIR)
    if key not in _NC_CACHE:
        _NC_CACHE[key] = build_nc()
    return _NC_CACHE[key]


def _host_dt():
    if _DT == "bf16":
        import ml_dtypes

        return ml_dtypes.bfloat16
    return np.float32


def shard_inputs(query_data, key_data, value_data, Wq, Wk, Wv, Wp):
    """Build the 8 per-core input maps."""
    hdt = _host_dt()
    qd = np.asarray(query_data, np.float32)
    kd = np.asarray(key_data, np.float32)
    vd = np.asarray(value_data, np.float32)
    Wqs = np.asarray(Wq, np.float32) * (1.0 / np.sqrt(DK))  # fold score scale into Wq
    Wk = np.asarray(Wk, np.float32)
    Wv = np.asarray(Wv, np.float32)
    Wp = np.asarray(Wp, np.float32)

    xqT = [np.ascontiguousarray(qd[b].T).astype(hdt) for b in range(B)]
    xkT = [np.ascontiguousarray(kd[b].T).astype(hdt) for b in range(B)]
    xvT = [np.ascontiguousarray(vd[b].T).astype(hdt) for b in range(B)]

    in_maps = []
    for c in range(NCORES):
        b, hg = divmod(c, HEAD_SHARDS)
        cs = slice(hg * DL, (hg + 1) * DL)
        in_maps.append(
            {
                "xq": xqT[b],
                "xk": xkT[b],
                "xv": xvT[b],
                "wq": np.ascontiguousarray(Wqs[:, cs]).astype(hdt),
                "wk": np.ascontiguousarray(Wk[:, cs]).astype(hdt),
                "wv": np.ascontiguousarray(Wv[:, cs]).astype(hdt),
                "wp": np.ascontiguousarray(Wp[cs, :]).astype(hdt),
            }
        )
    return in_maps


def kernel(query_data, key_data, value_data, Wq, Wk, Wv, Wp, bp):
    from concourse.bass_utils import run_bass_kernel_spmd

    nc = _get_nc()
    in_maps = shard_inputs(query_data, key_data, value_data, Wq, Wk, Wv, Wp)
    res = run_bass_kernel_spmd(nc, in_maps, list(range(NCORES))).results
    out = np.zeros((B, S, D), np.float32)
    for c in range(NCORES):
        b = c // HEAD_SHARDS
        out[b] += res[c]["out"].astype(np.float32)
    out += np.asarray(bp, np.float32)
    return out
